# revision 1
# baseline (speedup 1.0000x reference)
import numpy as np

# nn_AttentionPooling: pooled = segsum(softmax_seg(MLP(x)) * x) @ Wp + bp
# N=1M nodes, D=256, B=4096 segments, batch sorted. 8 NeuronCores.
#
# Strategy: shard nodes at segment boundaries so core c owns segments
# [512c, 512(c+1)) exactly -> the segment reduction is fully core-local and
# no collective is needed. Within a core, nodes are further split at every
# 128-segment boundary into 4 "groups"; each group accumulates a PSUM chunk
# U[128 segs, 256+1] via one-hot weighted matmuls (one-hot built on-device
# from host-precomputed relative segment ids). exp(s) is computed with a
# fixed offset C instead of the per-segment max (mathematically identical
# softmax; s is bounded by ||w2||_1 so no overflow).

N = 1_000_000
D = 256
B = 4096
NCORES = 8
SEGS_PER_CORE = B // NCORES          # 512
CHUNK = 128                          # segments per PSUM chunk
GROUPS = SEGS_PER_CORE // CHUNK      # 4
SUB = 128                            # nodes per subtile (partition dim)
SPS = 4                              # subtiles per super-tile
C_OFF = 4.0                          # exp(s - C_OFF) for range safety

_patched = False
WORK_FRAC = 1.0  # debug knob: fraction of super-tiles emitted (timing experiments)


def _patch_drain():
    """walrus core_v3 allows 1 sync-wait per CTRL drain; split Tile's tail
    drain waits across a chain of drains."""
    global _patched
    if _patched:
        return
    import concourse.tile as tile_mod

    def _split_drain_and_barrier(self, tick_clock, wait_clock):
        drain_inst = self.nc.sync.drain()
        wait_clock.add_sem_waits(
            drain_inst.ins, tile_mod.ScopedClock({None: tick_clock.global_clock})
        )
        si = drain_inst.ins.sync_info
        if si is not None and si.on_wait is not None and len(si.on_wait) > 1:
            waits = list(si.on_wait)
            SI = type(si)
            si.on_wait = waits[:1]
            for w in waits[1:]:
                extra = self.nc.sync.drain()
                extra.ins.sync_info = SI(on_wait=[w], on_update=[])
        self.nc.all_engine_barrier()
        assert self.sems is not None
        popped = self.nc._tile_sem_poison_stack.pop()
        assert popped is self._sem_poison
        self.nc.clear_and_free_semaphores(list(self.sems.allocated().values()))
        self.nc.all_engine_barrier()

    tile_mod.TileContext._drain_and_barrier = _split_drain_and_barrier

    # Split >1-wait instructions: walrus codegen has tiny per-instruction
    # sync-wait caps. Insert same-engine NOPs carrying the excess waits.
    import concourse.mybir as mybir
    _orig_lower = tile_mod.TileContext._lower_ordered_insts

    def _lower_with_wait_split(self, ordered):
        for bbname in list(ordered.keys()):
            insts = ordered[bbname]
            newl = []
            for inst in insts:
                si = getattr(inst, "sync_info", None)
                eng = getattr(inst, "engine", None)
                ow = list(si.on_wait) if (si is not None and si.on_wait) else []
                if (
                    len(ow) > 1
                    and eng is not None
                    and eng in self.nc.engines
                    and not isinstance(inst, tile_mod.TileBranchInst)
                ):
                    SI = type(si)
                    si.on_wait = ow[-1:]
                    for w in ow[:-1]:
                        nop = self.nc.engines[eng].nop(nofuse=True, hint="wsplit")
                        nop.ins.sync_info = SI(on_wait=[w], on_update=[])
                        newl.append(nop.ins)
                newl.append(inst)
            ordered[bbname] = newl
        return _orig_lower(self, ordered)

    tile_mod.TileContext._lower_ordered_insts = _lower_with_wait_split
    _patched = True


def _build_nc(n_super_per_group):
    import concourse.bass as bass
    import concourse.mybir as mybir
    from concourse.tile import TileContext

    dt = mybir.dt
    f32 = dt.float32
    f32r = dt.float32r
    bf16 = dt.bfloat16
    Alu = mybir.AluOpType
    Act = mybir.ActivationFunctionType

    SG = n_super_per_group
    n_super = GROUPS * SG
    n_sub = n_super * SPS
    n_nodes = n_sub * SUB

    nc = bass.Bass(target_bir_lowering=False, use_seq_codegen=True)

    xs = nc.declare_dram_parameter("xs", [n_super, SUB, 1032], f32r, isOutput=False)
    relT = nc.declare_dram_parameter("relT", [SUB, n_sub], f32, isOutput=False)
    w1sb_in = nc.declare_dram_parameter("w1sb", [128, 512], bf16, isOutput=False)
    b1c_in = nc.declare_dram_parameter("b1c", [128, 2], f32, isOutput=False)
    w2c_in = nc.declare_dram_parameter("w2c", [128, 2], bf16, isOutput=False)
    wpsb_in = nc.declare_dram_parameter("wpsb", [128, 512], f32r, isOutput=False)
    bpb_in = nc.declare_dram_parameter("bpb", [128, 256], f32, isOutput=False)
    iota_in = nc.declare_dram_parameter("iota", [128, 128], f32, isOutput=False)
    ebias_in = nc.declare_dram_parameter("ebias", [1, 1], f32, isOutput=False)
    idbf_in = nc.declare_dram_parameter("idbf", [128, 128], bf16, isOutput=False)
    idf_in = nc.declare_dram_parameter("idf", [128, 128], f32r, isOutput=False)
    out_sh = nc.declare_dram_parameter("out", [SEGS_PER_CORE, D], f32, isOutput=True)

    XA_W = SPS * 258          # per-super x_aug cols: 4 x [256 x | 2 ones]

    from contextlib import ExitStack
    with TileContext(nc) as tc:
        with ExitStack() as stk:
            ec = stk.enter_context
            cpool = ec(tc.tile_pool(name="consts", bufs=1))
            xapool = ec(tc.tile_pool(name="xa", bufs=10))
            xbfpool = ec(tc.tile_pool(name="xbf", bufs=12))
            sxtpool = ec(tc.tile_pool(name="sxt", bufs=8))
            thpool = ec(tc.tile_pool(name="th", bufs=8))
            erpool = ec(tc.tile_pool(name="erow", bufs=6))
            e4pool = ec(tc.tile_pool(name="e4", bufs=6))
            oepool = ec(tc.tile_pool(name="oe", bufs=12))
            relpool = ec(tc.tile_pool(name="rel", bufs=3))
            ufpool = ec(tc.tile_pool(name="uflush", bufs=2))
            sutpool = ec(tc.tile_pool(name="sut", bufs=2))
            rdpool = ec(tc.tile_pool(name="rd", bufs=2))
            osbpool = ec(tc.tile_pool(name="osb", bufs=2))
            # ---- constants into SBUF
            w1sb = cpool.tile([128, 512], bf16, tag="w1sb")
            nc.sync.dma_start(out=w1sb[:, :], in_=w1sb_in[:, :])
            b1c = cpool.tile([128, 2], f32, tag="b1c")
            nc.sync.dma_start(out=b1c[:, :], in_=b1c_in[:, :])
            w2c = cpool.tile([128, 2], bf16, tag="w2c")
            nc.sync.dma_start(out=w2c[:, :], in_=w2c_in[:, :])
            wpsb = cpool.tile([128, 512], f32r, tag="wpsb")
            nc.sync.dma_start(out=wpsb[:, :], in_=wpsb_in[:, :])
            bpb = cpool.tile([128, 256], f32, tag="bpb")
            nc.sync.dma_start(out=bpb[:, :], in_=bpb_in[:, :])
            iota = cpool.tile([128, 128], f32, tag="iota")
            nc.sync.dma_start(out=iota[:, :], in_=iota_in[:, :])
            idbf = cpool.tile([128, 128], bf16, tag="idbf")
            nc.sync.dma_start(out=idbf[:, :], in_=idbf_in[:, :])
            idf = cpool.tile([128, 128], f32r, tag="idf")
            nc.sync.dma_start(out=idf[:, :], in_=idf_in[:, :])
            ebias = cpool.tile([1, 1], f32, tag="ebias")
            nc.sync.dma_start(out=ebias[:, :], in_=ebias_in[:, :])

            pxtpool = ec(tc.tile_pool(name="pxt", bufs=2, space="PSUM"))
            phpool = ec(tc.tile_pool(name="ph", bufs=2, space="PSUM"))
            miscpool = ec(tc.tile_pool(name="misc", bufs=2, space="PSUM"))
            pupool = ec(tc.tile_pool(name="pu", bufs=2, space="PSUM"))
            if True:
                rel_sb = None
                SG_EFF = max(1, int(SG * WORK_FRAC))
                for g in range(GROUPS):
                    pu = pupool.tile([128, 258], f32, tag="pu")
                    rel_sb = relpool.tile([128, SG * SPS], f32, tag="rel")
                    nc.sync.dma_start(
                        out=rel_sb[:, :],
                        in_=relT[:, g * SG * SPS : (g + 1) * SG * SPS],
                    )
                    for it in range(SG_EFF):
                        sidx = g * SG + it           # super-tile index
                        xa = xapool.tile([128, XA_W], f32r, tag="xa")
                        nc.sync.dma_start(out=xa[:, :], in_=xs[sidx])
                        relbase = it * SPS

                        # cast x -> bf16 (gpsimd, 1-input ~line rate)
                        xbf = xbfpool.tile([128, SPS * 256], bf16, tag="xbf")
                        nc.gpsimd.tensor_copy(
                            out=xbf.rearrange("p (j c) -> p j c", c=256),
                            in_=xa.rearrange("p (j c) -> p j c", c=258)[:, :, 0:256],
                        )

                        # transpose x (PE) -> psum, copy to sbuf (DVE)
                        pxt = pxtpool.tile([128, SPS * 256], bf16, tag="pxt")
                        for j in range(SPS):
                            for k in range(2):
                                nc.tensor.transpose(
                                    pxt[:, k * 512 + j * 128 : k * 512 + (j + 1) * 128],
                                    xbf[:, j * 256 + k * 128 : j * 256 + (k + 1) * 128],
                                    idbf,
                                )
                        sxt = sxtpool.tile([128, SPS * 256], bf16, tag="sxt")
                        nc.vector.tensor_copy(out=sxt[:, :], in_=pxt[:, :])

                        # hT = W1^T x^T  (2 dout blocks x 2 k blocks)
                        ph0 = phpool.tile([128, 512], f32, tag="ph")
                        ph1 = phpool.tile([128, 512], f32, tag="ph")
                        for dblk, ph in ((0, ph0), (1, ph1)):
                            for k in range(2):
                                nc.tensor.matmul(
                                    ph[:, :],
                                    lhsT=w1sb[:, (2 * k + dblk) * 128 : (2 * k + dblk + 1) * 128],
                                    rhs=sxt[:, k * 512 : (k + 1) * 512],
                                    start=(k == 0),
                                    stop=(k == 1),
                                )
                        # tanh(h + b1)  (ACT, per-partition bias)
                        th0 = thpool.tile([128, 512], bf16, tag="th0")
                        th1 = thpool.tile([128, 512], bf16, tag="th1")
                        nc.scalar.activation(th0[:, :], ph0[:, :], Act.Tanh, bias=b1c[:, 0:1])
                        nc.scalar.activation(th1[:, :], ph1[:, :], Act.Tanh, bias=b1c[:, 1:2])

                        # s = th^T w2 -> [1, 512] psum
                        misc = miscpool.tile([128, 512], f32, tag="misc")
                        ps = misc[0:1, :]
                        nc.tensor.matmul(ps, lhsT=w2c[:, 0:1], rhs=th0[:, :], start=True, stop=False)
                        nc.tensor.matmul(ps, lhsT=w2c[:, 1:2], rhs=th1[:, :], start=False, stop=True)

                        # e = exp(s + b2 - C)
                        erow = erpool.tile([1, 512], f32, tag="erow")
                        nc.scalar.activation(erow[:, :], ps, Act.Exp, bias=ebias[0:1, 0:1])

                        # transpose e -> [128, 4]
                        pet = misc[:, 0:4]
                        for j in range(SPS):
                            nc.tensor.transpose(
                                pet[:, j : j + 1],
                                erow[0:1, j * 128 : (j + 1) * 128],
                                iota[0:1, 1:2],
                            )
                        e4 = e4pool.tile([128, 4], f32, tag="e4")
                        nc.vector.tensor_copy(out=e4[:, :], in_=pet)

                        # per subtile: Oe = (iota==rel) * e ; U += Oe^T @ [x|1]
                        for j in range(SPS):
                            oe = oepool.tile([128, 128], f32r, tag="oe")
                            nc.vector.tensor_scalar(
                                out=oe[:, :],
                                in0=iota[:, :],
                                scalar1=rel_sb[:, relbase + j : relbase + j + 1],
                                scalar2=e4[:, j : j + 1],
                                op0=Alu.is_equal,
                                op1=Alu.mult,
                            )
                            nc.tensor.matmul(
                                pu[:, :],
                                lhsT=oe[:, :],
                                rhs=xa[:, j * 258 : j * 258 + 258],
                                start=(it == 0 and j == 0),
                                stop=(it == SG_EFF - 1 and j == SPS - 1),
                                skip_group_check=True,
                            )
                    # flush group chunk to SBUF
                    uf = ufpool.tile([128, 258], f32r, tag="uf")
                    nc.vector.tensor_copy(out=uf[:, :], in_=pu[:, :])
                    # epilogue for this group: out = (U @ Wp) / denom + bp
                    put = pxtpool.tile([128, 256], f32r, tag="pxt")
                    nc.tensor.transpose(put[:, 0:128], uf[:, 0:128], idf)
                    nc.tensor.transpose(put[:, 128:256], uf[:, 128:256], idf)
                    sut = sutpool.tile([128, 256], f32r, tag="sut")
                    nc.vector.tensor_copy(out=sut[:, :], in_=put[:, :])
                    po = pupool.tile([128, 256], f32, tag="pu")
                    nc.tensor.matmul(po[:, :], lhsT=sut[:, 0:128], rhs=wpsb[:, 0:256], start=True, stop=False)
                    nc.tensor.matmul(po[:, :], lhsT=sut[:, 128:256], rhs=wpsb[:, 256:512], start=False, stop=True)
                    rd = rdpool.tile([128, 1], f32, tag="rd")
                    nc.vector.reciprocal(out=rd[:, :], in_=uf[:, 256:257])
                    osb = osbpool.tile([128, 256], f32, tag="osb")
                    nc.vector.scalar_tensor_tensor(
                        out=osb[:, :],
                        in0=po[:, :],
                        scalar=rd[:, 0:1],
                        in1=bpb[:, :],
                        op0=Alu.mult,
                        op1=Alu.add,
                    )
                    nc.sync.dma_start(
                        out=out_sh[g * 128 : (g + 1) * 128, :], in_=osb[:, :]
                    )
    return nc


def _prepare(x, batch, W1, b1, w2, b2, Wp, bp):
    import ml_dtypes

    _patch_drain()

    x = np.asarray(x, dtype=np.float32)
    batch_np = np.asarray(batch).astype(np.int64)
    W1 = np.asarray(W1, dtype=np.float32)
    b1 = np.asarray(b1, dtype=np.float32)
    w2 = np.asarray(w2, dtype=np.float32)
    b2 = float(np.asarray(b2))
    Wp = np.asarray(Wp, dtype=np.float32)
    bp = np.asarray(bp, dtype=np.float32)

    n, d = x.shape
    assert (n, d) == (N, D)

    # piece p (p = 0..31): nodes whose segment is in [128p, 128(p+1))
    bounds = np.searchsorted(batch_np, np.arange(0, B + 1, CHUNK))  # [33]
    piece_nodes = np.diff(bounds)
    SG = int(np.ceil(piece_nodes.max() / (SPS * SUB)))
    n_super = GROUPS * SG
    n_sub = n_super * SPS
    n_nodes_pad = n_sub * SUB

    nc = _build_nc(SG)

    # constant payloads (shared by all cores)
    w1sb = np.zeros((128, 512), dtype=ml_dtypes.bfloat16)
    for k in range(2):
        for dblk in range(2):
            w1sb[:, (2 * k + dblk) * 128 : (2 * k + dblk + 1) * 128] = (
                W1[k * 128 : (k + 1) * 128, dblk * 128 : (dblk + 1) * 128]
            ).astype(ml_dtypes.bfloat16)
    b1c = np.stack([b1[0:128], b1[128:256]], axis=1).astype(np.float32)
    w2c = np.stack([w2[0:128], w2[128:256]], axis=1).astype(ml_dtypes.bfloat16)
    wpsb = np.zeros((128, 512), dtype=np.float32)
    wpsb[:, 0:256] = Wp[0:128, :]
    wpsb[:, 256:512] = Wp[128:256, :]
    bpb = np.tile(bp[None, :], (128, 1)).astype(np.float32)
    iota = np.tile(np.arange(128, dtype=np.float32)[None, :], (128, 1))
    idbf = np.eye(128, dtype=ml_dtypes.bfloat16)
    idf = np.eye(128, dtype=np.float32)

    in_maps = []
    for c in range(NCORES):
        xflat = np.zeros((n_nodes_pad, D), dtype=np.float32)
        rel_c = np.full(n_sub * SUB, -1.0, dtype=np.float32)
        for g in range(GROUPS):
            p = c * GROUPS + g
            plo, phi = int(bounds[p]), int(bounds[p + 1])
            npc = phi - plo
            off = g * SG * SPS * SUB
            xflat[off : off + npc] = x[plo:phi]
            rel_c[off : off + npc] = (batch_np[plo:phi] - (p * CHUNK)).astype(
                np.float32
            )
        # device layout: [n_super, 128, 4*258] with cols 256,257 of each
        # 258-block = 1.0 (denominator ones)
        xs_c = np.ones((n_super, SUB, SPS, 258), dtype=np.float32)
        xs_c[:, :, :, 0:256] = xflat.reshape(n_super, SPS, SUB, D).transpose(
            0, 2, 1, 3
        )
        xs_c = np.ascontiguousarray(xs_c.reshape(n_super, SUB, SPS * 258))
        relT_c = np.ascontiguousarray(
            rel_c.reshape(n_sub, SUB).T
        )  # [128, n_sub]
        in_maps.append(
            {
                "xs": xs_c,
                "relT": relT_c,
                "w1sb": w1sb,
                "b1c": b1c,
                "w2c": w2c,
                "wpsb": wpsb,
                "bpb": bpb,
                "iota": iota,
                "ebias": np.array([[b2 - C_OFF]], dtype=np.float32),
                "idbf": idbf,
                "idf": idf,
            }
        )

    return nc, in_maps


def kernel(x, batch, W1, b1, w2, b2, Wp, bp):
    from concourse.bass_utils import run_bass_kernel_spmd

    nc, in_maps = _prepare(x, batch, W1, b1, w2, b2, Wp, bp)
    import kernel as _self
    res = run_bass_kernel_spmd(nc, in_maps, core_ids=list(range(NCORES)))
    _self._last_res = res
    out = np.concatenate([res.results[c]["out"] for c in range(NCORES)], axis=0)
    return out.astype(np.float32)



# revision 21
# speedup vs baseline: 78.2242x; 78.2242x over previous
import numpy as np

# nn_AttentionPooling: pooled = segsum(softmax_seg(MLP(x)) * x) @ Wp + bp
# N=1M nodes, D=256, B=4096 segments, batch sorted. 8 NeuronCores.
#
# Strategy: shard nodes at segment boundaries so core c owns segments
# [512c, 512(c+1)) exactly -> the segment reduction is fully core-local and
# no collective is needed. Within a core, nodes are further split at every
# 128-segment boundary into 4 "groups"; each group accumulates a PSUM chunk
# U[128 segs, 256+1] via one-hot weighted matmuls (one-hot built on-device
# from host-precomputed relative segment ids). exp(s) is computed with a
# fixed offset C instead of the per-segment max (mathematically identical
# softmax; s is bounded by ||w2||_1 so no overflow).
#
# v2: host ships BOTH x layouts in fp16 (natural for the pooling matmul,
# pre-transposed for the score MLP) -> no on-device transposes/casts.
# Scores come out column-per-subtile via free-size-1 matmuls
# (lhsT=tanh-block, rhs=w2-column), so exp runs on [128,4] not [1,512].
# v3: the natural layout is shipped only for SHIP_NUM/SHIP_DEN of the
# supers; the rest are PE-transposed from xT on device, balancing the
# DMA queue against PE/ACT. The denominator is a free-size-1 matmul
# against a ones column (no ones cols in the shipped layout).

N = 1_000_000
D = 256
B = 4096
NCORES = 8
SEGS_PER_CORE = B // NCORES          # 512
CHUNK = 128                          # segments per PSUM chunk
GROUPS = SEGS_PER_CORE // CHUNK      # 4
SUB = 128                            # nodes per subtile (partition dim)
SPS = 4                              # subtiles per super-tile
C_OFF = 4.0                          # exp(s - C_OFF) for range safety
SHIP_NUM, SHIP_DEN = 4, 5            # fraction of supers shipped in natural layout

_patched = False
WORK_FRAC = 1.0  # debug knob: fraction of super-tiles emitted (timing experiments)


def _patch_drain():
    """walrus core_v3 allows 1 sync-wait per CTRL drain; split Tile's tail
    drain waits across a chain of drains."""
    global _patched
    if _patched:
        return
    import concourse.tile as tile_mod

    def _split_drain_and_barrier(self, tick_clock, wait_clock):
        drain_inst = self.nc.sync.drain()
        wait_clock.add_sem_waits(
            drain_inst.ins, tile_mod.ScopedClock({None: tick_clock.global_clock})
        )
        si = drain_inst.ins.sync_info
        if si is not None and si.on_wait is not None and len(si.on_wait) > 1:
            waits = list(si.on_wait)
            SI = type(si)
            si.on_wait = waits[:1]
            for w in waits[1:]:
                extra = self.nc.sync.drain()
                extra.ins.sync_info = SI(on_wait=[w], on_update=[])
        self.nc.all_engine_barrier()
        assert self.sems is not None
        popped = self.nc._tile_sem_poison_stack.pop()
        assert popped is self._sem_poison
        self.nc.clear_and_free_semaphores(list(self.sems.allocated().values()))
        self.nc.all_engine_barrier()

    tile_mod.TileContext._drain_and_barrier = _split_drain_and_barrier

    # Split >1-wait instructions: walrus codegen has tiny per-instruction
    # sync-wait caps. Insert same-engine NOPs carrying the excess waits.
    import concourse.mybir as mybir
    _orig_lower = tile_mod.TileContext._lower_ordered_insts

    def _lower_with_wait_split(self, ordered):
        for bbname in list(ordered.keys()):
            insts = ordered[bbname]
            newl = []
            for inst in insts:
                si = getattr(inst, "sync_info", None)
                eng = getattr(inst, "engine", None)
                ow = list(si.on_wait) if (si is not None and si.on_wait) else []
                if (
                    len(ow) > 1
                    and eng is not None
                    and eng in self.nc.engines
                    and not isinstance(inst, tile_mod.TileBranchInst)
                ):
                    SI = type(si)
                    si.on_wait = ow[-1:]
                    for w in ow[:-1]:
                        nop = self.nc.engines[eng].nop(nofuse=True, hint="wsplit")
                        nop.ins.sync_info = SI(on_wait=[w], on_update=[])
                        newl.append(nop.ins)
                newl.append(inst)
            ordered[bbname] = newl
        return _orig_lower(self, ordered)

    tile_mod.TileContext._lower_ordered_insts = _lower_with_wait_split
    _patched = True


def _tr_set(SG, n_ship):
    """Positions of device-transposed supers: spread every 4th slot so the
    PE-heavy transpose work interleaves with DMA-heavy shipped supers."""
    n_tr = SG - n_ship
    slots = [i for i in range(SG) if i % 4 == 1]
    if len(slots) < n_tr:
        slots += [i for i in range(SG) if i % 4 == 3]
    return set(slots[:n_tr])


def _build_nc(n_super_per_group):
    import concourse.bass as bass
    import concourse.mybir as mybir
    from concourse.tile import TileContext

    dt = mybir.dt
    f32 = dt.float32
    f32r = dt.float32r
    f16 = dt.float16
    Alu = mybir.AluOpType
    Act = mybir.ActivationFunctionType

    SG = n_super_per_group
    n_super = GROUPS * SG
    n_sub = n_super * SPS

    nc = bass.Bass(target_bir_lowering=False, use_seq_codegen=True)

    XA_W = SPS * 256          # per-super natural-x cols: 4 x 256
    XT_W = 2 * SPS * SUB      # per-super xT cols: 2 k-blocks x 512 nodes
    n_ship = (SG * SHIP_NUM + SHIP_DEN - 1) // SHIP_DEN

    xs = nc.declare_dram_parameter("xs", [GROUPS * n_ship, SUB, XA_W], f16, isOutput=False)
    xts = nc.declare_dram_parameter("xts", [n_super, SUB, XT_W], f16, isOutput=False)
    relT = nc.declare_dram_parameter("relT", [SUB, n_sub], f32, isOutput=False)
    w1sb_in = nc.declare_dram_parameter("w1sb", [128, 512], f16, isOutput=False)
    b1c_in = nc.declare_dram_parameter("b1c", [128, 2], f32, isOutput=False)
    w2c_in = nc.declare_dram_parameter("w2c", [128, 2], f16, isOutput=False)
    wpsb_in = nc.declare_dram_parameter("wpsb", [128, 512], f32, isOutput=False)
    bpb_in = nc.declare_dram_parameter("bpb", [128, 256], f32, isOutput=False)
    iota_in = nc.declare_dram_parameter("iota", [128, 128], f16, isOutput=False)
    idh_in = nc.declare_dram_parameter("idh", [128, 128], f16, isOutput=False)
    ones_in = nc.declare_dram_parameter("ones", [128, 1], f16, isOutput=False)
    ebias_in = nc.declare_dram_parameter("ebias", [128, 1], f32, isOutput=False)
    idf_in = nc.declare_dram_parameter("idf", [128, 128], f32, isOutput=False)
    out_sh = nc.declare_dram_parameter("out", [SEGS_PER_CORE, D], f32, isOutput=True)

    from contextlib import ExitStack
    with TileContext(nc) as tc:
        with ExitStack() as stk:
            ec = stk.enter_context
            cpool = ec(tc.tile_pool(name="consts", bufs=1))
            xapool = ec(tc.tile_pool(name="xa", bufs=18))
            xtpool = ec(tc.tile_pool(name="xt", bufs=20))
            sxtpool = ec(tc.tile_pool(name="sxt", bufs=6))
            thpool = ec(tc.tile_pool(name="th", bufs=10))
            e4pool = ec(tc.tile_pool(name="e4", bufs=6))
            oepool = ec(tc.tile_pool(name="oe", bufs=24))
            relpool = ec(tc.tile_pool(name="rel", bufs=3))
            ufpool = ec(tc.tile_pool(name="uflush", bufs=2))
            sutpool = ec(tc.tile_pool(name="sut", bufs=2))
            rdpool = ec(tc.tile_pool(name="rd", bufs=2))
            osbpool = ec(tc.tile_pool(name="osb", bufs=2))
            # ---- constants into SBUF
            w1sb = cpool.tile([128, 512], f16, tag="w1sb")
            nc.sync.dma_start(out=w1sb[:, :], in_=w1sb_in[:, :])
            b1c = cpool.tile([128, 2], f32, tag="b1c")
            nc.sync.dma_start(out=b1c[:, :], in_=b1c_in[:, :])
            w2c = cpool.tile([128, 2], f16, tag="w2c")
            nc.sync.dma_start(out=w2c[:, :], in_=w2c_in[:, :])
            wpsb = cpool.tile([128, 512], f32, tag="wpsb")
            nc.sync.dma_start(out=wpsb[:, :], in_=wpsb_in[:, :])
            bpb = cpool.tile([128, 256], f32, tag="bpb")
            nc.sync.dma_start(out=bpb[:, :], in_=bpb_in[:, :])
            iota = cpool.tile([128, 128], f16, tag="iota")
            nc.sync.dma_start(out=iota[:, :], in_=iota_in[:, :])
            idh = cpool.tile([128, 128], f16, tag="idh")
            nc.sync.dma_start(out=idh[:, :], in_=idh_in[:, :])
            onescol = cpool.tile([128, 1], f16, tag="ones")
            nc.sync.dma_start(out=onescol[:, :], in_=ones_in[:, :])
            idf = cpool.tile([128, 128], f32, tag="idf")
            nc.sync.dma_start(out=idf[:, :], in_=idf_in[:, :])
            ebias = cpool.tile([128, 1], f32, tag="ebias")
            nc.sync.dma_start(out=ebias[:, :], in_=ebias_in[:, :])

            phpool = ec(tc.tile_pool(name="ph", bufs=3, space="PSUM"))
            pupool = ec(tc.tile_pool(name="pu", bufs=2, space="PSUM"))
            eppool = ec(tc.tile_pool(name="ep", bufs=1, space="PSUM"))
            pxtpool = ec(tc.tile_pool(name="pxt", bufs=1, space="PSUM"))
            ps4pool = ec(tc.tile_pool(name="ps4", bufs=1, space="PSUM"))

            SG_EFF = max(1, int(SG * WORK_FRAC))
            for g in range(GROUPS):
                pu = pupool.tile([128, 257], f32, tag="pu")
                ps4b = ps4pool.tile([128, 16], f32, tag="ps4")
                rel_sb = relpool.tile([128, SG * SPS], f32, tag="rel")
                nc.sync.dma_start(
                    out=rel_sb[:, :],
                    in_=relT[:, g * SG * SPS : (g + 1) * SG * SPS],
                )
                trs = _tr_set(SG, n_ship) if SG_EFF == SG else set()
                xnats = [None] * 4
                last_flushed = -1
                ship_i = 0
                for it in range(SG_EFF):
                    sidx = g * SG + it           # super-tile index
                    xt = xtpool.tile([128, XT_W], f16, tag="xt")
                    nc.sync.dma_start(out=xt[:, :], in_=xts[sidx])
                    if it not in trs:
                        xa = xapool.tile([128, XA_W], f16, tag="xa")
                        nc.sync.dma_start(out=xa[:, :], in_=xs[g * n_ship + ship_i])
                        ship_i += 1
                        xnats[it % 4] = xa
                    else:
                        pxt = pxtpool.tile([128, XA_W], f16, tag="pxt")
                        for j in range(SPS):
                            for kb in range(2):
                                nc.tensor.transpose(
                                    pxt[:, j * 256 + kb * 128 : j * 256 + (kb + 1) * 128],
                                    xt[:, kb * 512 + j * 128 : kb * 512 + (j + 1) * 128],
                                    idh,
                                )
                        sxt = sxtpool.tile([128, XA_W], f16, tag="sxt")
                        nc.vector.tensor_copy(out=sxt[:, :], in_=pxt[:, :])
                        xnats[it % 4] = sxt

                    # hT = W1^T x^T  (2 dout blocks x 2 k blocks)
                    ph0 = phpool.tile([128, 512], f32, tag="ph")
                    ph1 = phpool.tile([128, 512], f32, tag="ph")
                    for dblk, ph in ((0, ph0), (1, ph1)):
                        for k in range(2):
                            nc.tensor.matmul(
                                ph[:, :],
                                lhsT=w1sb[:, (2 * k + dblk) * 128 : (2 * k + dblk + 1) * 128],
                                rhs=xt[:, k * 512 : (k + 1) * 512],
                                start=(k == 0),
                                stop=(k == 1),
                            )
                    # tanh(h + b1)  (ACT, per-partition bias)
                    th0 = thpool.tile([128, 512], f16, tag="th0")
                    th1 = thpool.tile([128, 512], f16, tag="th1")
                    nc.scalar.activation(th0[:, :], ph0[:, :], Act.Tanh, bias=b1c[:, 0:1])
                    nc.scalar.activation(th1[:, :], ph1[:, :], Act.Tanh, bias=b1c[:, 1:2])

                    # s^T columns: ps4[node, j] = sum_dout th[dout, node] w2[dout]
                    # (own PSUM bank - a start=True matmul marks its whole 2KB
                    # zero-region pending, so it must not share a bank with the
                    # long-lived pu accumulator; 4 regions, one exp per 4 supers)
                    ps4 = ps4b[:, 4 * (it % 4) : 4 * (it % 4) + 4]
                    for j in range(SPS):
                        nc.tensor.matmul(
                            ps4[:, j : j + 1],
                            lhsT=th0[:, j * 128 : (j + 1) * 128],
                            rhs=w2c[:, 0:1],
                            start=True,
                            stop=False,
                            skip_group_check=True,
                        )
                        nc.tensor.matmul(
                            ps4[:, j : j + 1],
                            lhsT=th1[:, j * 128 : (j + 1) * 128],
                            rhs=w2c[:, 1:2],
                            start=False,
                            stop=True,
                            skip_group_check=True,
                        )

                    # e = exp(s + b2 - C), batched over up to 4 supers;
                    # flush per-super near the group end to keep the epilogue
                    # off the batched critical path
                    if it % 4 == 3 or it >= SG_EFF - 4:
                        b0 = last_flushed + 1
                        e4b = e4pool.tile([128, 4 * SPS], f32, tag="e4")
                        nc.scalar.activation(
                            e4b[:, 4 * (b0 % 4) : 4 * (it % 4) + 4],
                            ps4b[:, 4 * (b0 % 4) : 4 * (it % 4) + 4],
                            Act.Exp,
                            bias=ebias[:, 0:1],
                        )
                        pend = list(range(b0, it + 1))
                        last_flushed = it
                    else:
                        pend = None

                    # per subtile: Oe = (iota==rel) * e ; U += Oe^T @ [x|1]
                    if pend is None:
                        continue
                    for pit in pend:
                        xnat = xnats[pit % 4]
                        relbase = pit * SPS
                        for j in range(SPS):
                            oe = oepool.tile([128, 128], f16, tag="oe")
                            nc.vector.tensor_scalar(
                                out=oe[:, :],
                                in0=iota[:, :],
                                scalar1=rel_sb[:, relbase + j : relbase + j + 1],
                                scalar2=e4b[:, 4 * (pit % 4) + j : 4 * (pit % 4) + j + 1],
                                op0=Alu.is_equal,
                                op1=Alu.mult,
                            )
                            nc.tensor.matmul(
                                pu[:, 0:256],
                                lhsT=oe[:, :],
                                rhs=xnat[:, j * 256 : (j + 1) * 256],
                                start=(pit == 0 and j == 0),
                                stop=(pit == SG_EFF - 1 and j == SPS - 1),
                                skip_group_check=True,
                            )
                            # start only on the group's first x-matmul: a
                            # start=True marks the whole 2KB zero-region
                            # pending, so a second start here would make the
                            # next x-matmul overwrite instead of accumulate.
                            nc.tensor.matmul(
                                pu[:, 256:257],
                                lhsT=oe[:, :],
                                rhs=onescol[:, 0:1],
                                start=False,
                                stop=(pit == SG_EFF - 1 and j == SPS - 1),
                                skip_group_check=True,
                            )
                # flush group chunk to SBUF
                uf = ufpool.tile([128, 257], f32, tag="uf")
                nc.vector.tensor_copy(out=uf[:, :], in_=pu[:, 0:257])
                # epilogue for this group: out = (U @ Wp) / denom + bp
                ep = eppool.tile([128, 512], f32, tag="ep")
                put = ep[:, 0:256]
                nc.tensor.transpose(put[:, 0:128], uf[:, 0:128], idf)
                nc.tensor.transpose(put[:, 128:256], uf[:, 128:256], idf)
                sut = sutpool.tile([128, 256], f32, tag="sut")
                nc.vector.tensor_copy(out=sut[:, :], in_=put[:, :])
                po = ep[:, 256:512]
                nc.tensor.matmul(po[:, :], lhsT=sut[:, 0:128], rhs=wpsb[:, 0:256], start=True, stop=False)
                nc.tensor.matmul(po[:, :], lhsT=sut[:, 128:256], rhs=wpsb[:, 256:512], start=False, stop=True)
                rd = rdpool.tile([128, 1], f32, tag="rd")
                nc.vector.reciprocal(out=rd[:, :], in_=uf[:, 256:257])
                osb = osbpool.tile([128, 256], f32, tag="osb")
                nc.vector.scalar_tensor_tensor(
                    out=osb[:, :],
                    in0=po[:, :],
                    scalar=rd[:, 0:1],
                    in1=bpb[:, :],
                    op0=Alu.mult,
                    op1=Alu.add,
                )
                nc.sync.dma_start(
                    out=out_sh[g * 128 : (g + 1) * 128, :], in_=osb[:, :]
                )
    return nc


def _prepare(x, batch, W1, b1, w2, b2, Wp, bp):
    _patch_drain()

    x = np.asarray(x, dtype=np.float32)
    batch_np = np.asarray(batch).astype(np.int64)
    W1 = np.asarray(W1, dtype=np.float32)
    b1 = np.asarray(b1, dtype=np.float32)
    w2 = np.asarray(w2, dtype=np.float32)
    b2 = float(np.asarray(b2))
    Wp = np.asarray(Wp, dtype=np.float32)
    bp = np.asarray(bp, dtype=np.float32)

    n, d = x.shape
    assert (n, d) == (N, D)

    # piece p (p = 0..31): nodes whose segment is in [128p, 128(p+1))
    bounds = np.searchsorted(batch_np, np.arange(0, B + 1, CHUNK))  # [33]
    piece_nodes = np.diff(bounds)
    SG = int(np.ceil(piece_nodes.max() / (SPS * SUB)))
    n_super = GROUPS * SG
    n_sub = n_super * SPS
    n_nodes_pad = n_sub * SUB

    n_ship = (SG * SHIP_NUM + SHIP_DEN - 1) // SHIP_DEN
    nc = _build_nc(SG)

    f16 = np.float16
    # constant payloads (shared by all cores)
    w1sb = np.zeros((128, 512), dtype=f16)
    for k in range(2):
        for dblk in range(2):
            w1sb[:, (2 * k + dblk) * 128 : (2 * k + dblk + 1) * 128] = (
                W1[k * 128 : (k + 1) * 128, dblk * 128 : (dblk + 1) * 128]
            ).astype(f16)
    b1c = np.stack([b1[0:128], b1[128:256]], axis=1).astype(np.float32)
    w2c = np.stack([w2[0:128], w2[128:256]], axis=1).astype(f16)
    wpsb = np.zeros((128, 512), dtype=np.float32)
    wpsb[:, 0:256] = Wp[0:128, :]
    wpsb[:, 256:512] = Wp[128:256, :]
    bpb = np.tile(bp[None, :], (128, 1)).astype(np.float32)
    iota = np.tile(np.arange(128, dtype=f16)[None, :], (128, 1))
    idf = np.eye(128, dtype=np.float32)
    idh = np.eye(128, dtype=f16)

    x16 = x.astype(f16)

    in_maps = []
    for c in range(NCORES):
        xflat = np.zeros((n_nodes_pad, D), dtype=f16)
        rel_c = np.full(n_sub * SUB, -1.0, dtype=np.float32)
        for g in range(GROUPS):
            p = c * GROUPS + g
            plo, phi = int(bounds[p]), int(bounds[p + 1])
            npc = phi - plo
            off = g * SG * SPS * SUB
            xflat[off : off + npc] = x16[plo:phi]
            rel_c[off : off + npc] = (batch_np[plo:phi] - (p * CHUNK)).astype(np.float32)
        # xa layout: [GROUPS*n_ship, 128, 4*256] natural-x, shipped supers only
        xnat_all = xflat.reshape(n_super, SPS, SUB, D).transpose(0, 2, 1, 3)
        trs = _tr_set(SG, n_ship)
        ship_idx = [g * SG + it for g in range(GROUPS) for it in range(SG) if it not in trs]
        xs_c = np.ascontiguousarray(
            xnat_all[ship_idx].reshape(GROUPS * n_ship, SUB, SPS * D)
        )
        # xT layout: [n_super, 128, 2*512]: row p, col k*512+n = x[node n, din 128k+p]
        xts_c = np.ascontiguousarray(
            xflat.reshape(n_super, SPS * SUB, 2, 128)
            .transpose(0, 3, 2, 1)
            .reshape(n_super, 128, 2 * SPS * SUB)
        )
        relT_c = np.ascontiguousarray(
            rel_c.reshape(n_sub, SUB).T
        )  # [128, n_sub]
        in_maps.append(
            {
                "xs": xs_c,
                "xts": xts_c,
                "relT": relT_c,
                "w1sb": w1sb,
                "b1c": b1c,
                "w2c": w2c,
                "wpsb": wpsb,
                "bpb": bpb,
                "iota": iota,
                "idf": idf,
                "idh": idh,
                "ones": np.ones((128, 1), dtype=f16),
                "ebias": np.full((128, 1), b2 - C_OFF, dtype=np.float32),
            }
        )

    return nc, in_maps


def kernel(x, batch, W1, b1, w2, b2, Wp, bp):
    from concourse.bass_utils import run_bass_kernel_spmd

    nc, in_maps = _prepare(x, batch, W1, b1, w2, b2, Wp, bp)
    import kernel as _self
    res = run_bass_kernel_spmd(nc, in_maps, core_ids=list(range(NCORES)))
    _self._last_res = res
    out = np.concatenate([res.results[c]["out"] for c in range(NCORES)], axis=0)
    return out.astype(np.float32)


# revision 24
# speedup vs baseline: 80.2944x; 1.0265x over previous
import numpy as np

# nn_AttentionPooling: pooled = segsum(softmax_seg(MLP(x)) * x) @ Wp + bp
# N=1M nodes, D=256, B=4096 segments, batch sorted. 8 NeuronCores.
#
# Strategy: shard nodes at segment boundaries so core c owns segments
# [512c, 512(c+1)) exactly -> the segment reduction is fully core-local and
# no collective is needed. Within a core, nodes are further split at every
# 128-segment boundary into 4 "groups"; each group accumulates a PSUM chunk
# U[128 segs, 256+1] via one-hot weighted matmuls (one-hot built on-device
# from host-precomputed relative segment ids). exp(s) is computed with a
# fixed offset C instead of the per-segment max (mathematically identical
# softmax; s is bounded by ||w2||_1 so no overflow).
#
# v2: host ships BOTH x layouts in fp16 (natural for the pooling matmul,
# pre-transposed for the score MLP) -> no on-device transposes/casts.
# Scores come out column-per-subtile via free-size-1 matmuls
# (lhsT=tanh-block, rhs=w2-column), so exp runs on [128,4] not [1,512].
# v3: the natural layout is shipped only for SHIP_NUM/SHIP_DEN of the
# supers; the rest are PE-transposed from xT on device, balancing the
# DMA queue against PE/ACT. The denominator is a free-size-1 matmul
# against a ones column (no ones cols in the shipped layout).

N = 1_000_000
D = 256
B = 4096
NCORES = 8
SEGS_PER_CORE = B // NCORES          # 512
CHUNK = 128                          # segments per PSUM chunk
GROUPS = SEGS_PER_CORE // CHUNK      # 4
SUB = 128                            # nodes per subtile (partition dim)
SPS = 4                              # subtiles per super-tile
C_OFF = 4.0                          # exp(s - C_OFF) for range safety
SHIP_NUM, SHIP_DEN = 4, 5            # fraction of supers shipped in natural layout

_patched = False
WORK_FRAC = 1.0  # debug knob: fraction of super-tiles emitted (timing experiments)


def _patch_drain():
    """walrus core_v3 allows 1 sync-wait per CTRL drain; split Tile's tail
    drain waits across a chain of drains."""
    global _patched
    if _patched:
        return
    import concourse.tile as tile_mod

    def _split_drain_and_barrier(self, tick_clock, wait_clock):
        drain_inst = self.nc.sync.drain()
        wait_clock.add_sem_waits(
            drain_inst.ins, tile_mod.ScopedClock({None: tick_clock.global_clock})
        )
        si = drain_inst.ins.sync_info
        if si is not None and si.on_wait is not None and len(si.on_wait) > 1:
            waits = list(si.on_wait)
            SI = type(si)
            si.on_wait = waits[:1]
            for w in waits[1:]:
                extra = self.nc.sync.drain()
                extra.ins.sync_info = SI(on_wait=[w], on_update=[])
        self.nc.all_engine_barrier()
        assert self.sems is not None
        popped = self.nc._tile_sem_poison_stack.pop()
        assert popped is self._sem_poison
        self.nc.clear_and_free_semaphores(list(self.sems.allocated().values()))
        self.nc.all_engine_barrier()

    tile_mod.TileContext._drain_and_barrier = _split_drain_and_barrier

    # Split >1-wait instructions: walrus codegen has tiny per-instruction
    # sync-wait caps. Insert same-engine NOPs carrying the excess waits.
    import concourse.mybir as mybir
    _orig_lower = tile_mod.TileContext._lower_ordered_insts

    def _lower_with_wait_split(self, ordered):
        for bbname in list(ordered.keys()):
            insts = ordered[bbname]
            newl = []
            for inst in insts:
                si = getattr(inst, "sync_info", None)
                eng = getattr(inst, "engine", None)
                ow = list(si.on_wait) if (si is not None and si.on_wait) else []
                if (
                    len(ow) > 1
                    and eng is not None
                    and eng in self.nc.engines
                    and not isinstance(inst, tile_mod.TileBranchInst)
                ):
                    SI = type(si)
                    si.on_wait = ow[-1:]
                    for w in ow[:-1]:
                        nop = self.nc.engines[eng].nop(nofuse=True, hint="wsplit")
                        nop.ins.sync_info = SI(on_wait=[w], on_update=[])
                        newl.append(nop.ins)
                newl.append(inst)
            ordered[bbname] = newl
        return _orig_lower(self, ordered)

    tile_mod.TileContext._lower_ordered_insts = _lower_with_wait_split
    _patched = True


def _tr_set(SG, n_ship):
    """Positions of device-transposed supers: spread every 4th slot so the
    PE-heavy transpose work interleaves with DMA-heavy shipped supers."""
    n_tr = SG - n_ship
    slots = [i for i in range(SG) if i % 4 == 1]
    if len(slots) < n_tr:
        slots += [i for i in range(SG) if i % 4 == 3]
    return set(slots[:n_tr])


def _build_nc(n_super_per_group):
    import concourse.bass as bass
    import concourse.mybir as mybir
    from concourse.tile import TileContext

    dt = mybir.dt
    f32 = dt.float32
    f32r = dt.float32r
    f16 = dt.float16
    Alu = mybir.AluOpType
    Act = mybir.ActivationFunctionType

    SG = n_super_per_group
    n_super = GROUPS * SG
    n_sub = n_super * SPS

    nc = bass.Bass(target_bir_lowering=False, use_seq_codegen=True)

    XA_W = SPS * 256          # per-super natural-x cols: 4 x 256
    XT_W = 2 * SPS * SUB      # per-super xT cols: 2 k-blocks x 512 nodes
    n_ship = (SG * SHIP_NUM + SHIP_DEN - 1) // SHIP_DEN

    xs = nc.declare_dram_parameter("xs", [GROUPS * n_ship, SUB, XA_W], f16, isOutput=False)
    xts = nc.declare_dram_parameter("xts", [n_super, SUB, XT_W], f16, isOutput=False)
    relT = nc.declare_dram_parameter("relT", [SUB, n_sub], f32, isOutput=False)
    w1sb_in = nc.declare_dram_parameter("w1sb", [128, 512], f16, isOutput=False)
    b1c_in = nc.declare_dram_parameter("b1c", [128, 2], f32, isOutput=False)
    w2c_in = nc.declare_dram_parameter("w2c", [128, 2], f16, isOutput=False)
    wpsb_in = nc.declare_dram_parameter("wpsb", [128, 512], f32, isOutput=False)
    bpb_in = nc.declare_dram_parameter("bpb", [128, 256], f32, isOutput=False)
    iota_in = nc.declare_dram_parameter("iota", [128, 128], f16, isOutput=False)
    idh_in = nc.declare_dram_parameter("idh", [128, 128], f16, isOutput=False)
    ones_in = nc.declare_dram_parameter("ones", [128, 1], f16, isOutput=False)
    ebias_in = nc.declare_dram_parameter("ebias", [128, 1], f32, isOutput=False)
    idf_in = nc.declare_dram_parameter("idf", [128, 128], f32, isOutput=False)
    out_sh = nc.declare_dram_parameter("out", [SEGS_PER_CORE, D], f32, isOutput=True)

    from contextlib import ExitStack
    with TileContext(nc) as tc:
        with ExitStack() as stk:
            ec = stk.enter_context
            cpool = ec(tc.tile_pool(name="consts", bufs=1))
            xapool = ec(tc.tile_pool(name="xa", bufs=18))
            xtpool = ec(tc.tile_pool(name="xt", bufs=20))
            sxtpool = ec(tc.tile_pool(name="sxt", bufs=6))
            thpool = ec(tc.tile_pool(name="th", bufs=10))
            e4pool = ec(tc.tile_pool(name="e4", bufs=6))
            oepool = ec(tc.tile_pool(name="oe", bufs=24))
            relpool = ec(tc.tile_pool(name="rel", bufs=3))
            ufpool = ec(tc.tile_pool(name="uflush", bufs=2))
            sutpool = ec(tc.tile_pool(name="sut", bufs=2))
            rdpool = ec(tc.tile_pool(name="rd", bufs=2))
            osbpool = ec(tc.tile_pool(name="osb", bufs=2))
            # ---- constants into SBUF
            w1sb = cpool.tile([128, 512], f16, tag="w1sb")
            nc.sync.dma_start(out=w1sb[:, :], in_=w1sb_in[:, :])
            b1c = cpool.tile([128, 2], f32, tag="b1c")
            nc.sync.dma_start(out=b1c[:, :], in_=b1c_in[:, :])
            w2c = cpool.tile([128, 2], f16, tag="w2c")
            nc.sync.dma_start(out=w2c[:, :], in_=w2c_in[:, :])
            wpsb = cpool.tile([128, 512], f32, tag="wpsb")
            nc.sync.dma_start(out=wpsb[:, :], in_=wpsb_in[:, :])
            bpb = cpool.tile([128, 256], f32, tag="bpb")
            nc.sync.dma_start(out=bpb[:, :], in_=bpb_in[:, :])
            iota = cpool.tile([128, 128], f16, tag="iota")
            nc.sync.dma_start(out=iota[:, :], in_=iota_in[:, :])
            idh = cpool.tile([128, 128], f16, tag="idh")
            nc.sync.dma_start(out=idh[:, :], in_=idh_in[:, :])
            onescol = cpool.tile([128, 1], f16, tag="ones")
            nc.sync.dma_start(out=onescol[:, :], in_=ones_in[:, :])
            idf = cpool.tile([128, 128], f32, tag="idf")
            nc.sync.dma_start(out=idf[:, :], in_=idf_in[:, :])
            ebias = cpool.tile([128, 1], f32, tag="ebias")
            nc.sync.dma_start(out=ebias[:, :], in_=ebias_in[:, :])

            phpool = ec(tc.tile_pool(name="ph", bufs=3, space="PSUM"))
            pupool = ec(tc.tile_pool(name="pu", bufs=2, space="PSUM"))
            eppool = ec(tc.tile_pool(name="ep", bufs=1, space="PSUM"))
            pxtpool = ec(tc.tile_pool(name="pxt", bufs=1, space="PSUM"))
            ps4pool = ec(tc.tile_pool(name="ps4", bufs=1, space="PSUM"))

            SG_EFF = max(1, int(SG * WORK_FRAC))
            for g in range(GROUPS):
                pu = pupool.tile([128, 257], f32, tag="pu")
                ps4b = ps4pool.tile([128, 16], f32, tag="ps4")
                rel_sb = relpool.tile([128, SG * SPS], f32, tag="rel")
                nc.sync.dma_start(
                    out=rel_sb[:, :],
                    in_=relT[:, g * SG * SPS : (g + 1) * SG * SPS],
                )
                trs = _tr_set(SG, n_ship) if SG_EFF == SG else set()
                xnats = [None] * 4
                last_flushed = -1
                ship_i = 0
                for it in range(SG_EFF):
                    sidx = g * SG + it           # super-tile index
                    xt = xtpool.tile([128, XT_W], f16, tag="xt")
                    nc.sync.dma_start(out=xt[:, :], in_=xts[sidx])
                    if it not in trs:
                        xa = xapool.tile([128, XA_W], f16, tag="xa")
                        nc.sync.dma_start(out=xa[:, :], in_=xs[g * n_ship + ship_i])
                        ship_i += 1
                        xnats[it % 4] = xa
                    else:
                        pxt = pxtpool.tile([128, XA_W], f16, tag="pxt")
                        for j in range(SPS):
                            for kb in range(2):
                                nc.tensor.transpose(
                                    pxt[:, j * 256 + kb * 128 : j * 256 + (kb + 1) * 128],
                                    xt[:, kb * 512 + j * 128 : kb * 512 + (j + 1) * 128],
                                    idh,
                                )
                        sxt = sxtpool.tile([128, XA_W], f16, tag="sxt")
                        nc.vector.tensor_copy(out=sxt[:, :], in_=pxt[:, :])
                        xnats[it % 4] = sxt

                    # hT = W1^T x^T  (2 dout blocks x 2 k blocks)
                    ph0 = phpool.tile([128, 512], f32, tag="ph")
                    ph1 = phpool.tile([128, 512], f32, tag="ph")
                    for dblk, ph in ((0, ph0), (1, ph1)):
                        for k in range(2):
                            nc.tensor.matmul(
                                ph[:, :],
                                lhsT=w1sb[:, (2 * k + dblk) * 128 : (2 * k + dblk + 1) * 128],
                                rhs=xt[:, k * 512 : (k + 1) * 512],
                                start=(k == 0),
                                stop=(k == 1),
                            )
                    # tanh(h + b1)  (ACT, per-partition bias)
                    th0 = thpool.tile([128, 512], f16, tag="th0")
                    th1 = thpool.tile([128, 512], f16, tag="th1")
                    nc.scalar.activation(th0[:, :], ph0[:, :], Act.Tanh, bias=b1c[:, 0:1])
                    nc.scalar.activation(th1[:, :], ph1[:, :], Act.Tanh, bias=b1c[:, 1:2])

                    # s^T columns: ps4[node, j] = sum_dout th[dout, node] w2[dout]
                    # (own PSUM bank - a start=True matmul marks its whole 2KB
                    # zero-region pending, so it must not share a bank with the
                    # long-lived pu accumulator; 4 regions, one exp per 4 supers)
                    ps4 = ps4b[:, 4 * (it % 4) : 4 * (it % 4) + 4]
                    for j in range(SPS):
                        nc.tensor.matmul(
                            ps4[:, j : j + 1],
                            lhsT=th0[:, j * 128 : (j + 1) * 128],
                            rhs=w2c[:, 0:1],
                            start=True,
                            stop=False,
                            skip_group_check=True,
                        )
                        nc.tensor.matmul(
                            ps4[:, j : j + 1],
                            lhsT=th1[:, j * 128 : (j + 1) * 128],
                            rhs=w2c[:, 1:2],
                            start=False,
                            stop=True,
                            skip_group_check=True,
                        )

                    # e = exp(s + b2 - C), batched over up to 4 supers;
                    # flush per-super near the group end to keep the epilogue
                    # off the batched critical path
                    if it % 4 == 3 or it >= SG_EFF - 4:
                        b0 = last_flushed + 1
                        e4b = e4pool.tile([128, 4 * SPS], f32, tag="e4")
                        nc.scalar.activation(
                            e4b[:, 4 * (b0 % 4) : 4 * (it % 4) + 4],
                            ps4b[:, 4 * (b0 % 4) : 4 * (it % 4) + 4],
                            Act.Exp,
                            bias=ebias[:, 0:1],
                        )
                        pend = list(range(b0, it + 1))
                        last_flushed = it
                    else:
                        pend = None

                    # per subtile: Oe = (iota==rel) * e ; U += Oe^T @ [x|1]
                    if pend is None:
                        continue
                    for pit in pend:
                        xnat = xnats[pit % 4]
                        relbase = pit * SPS
                        for j in range(SPS):
                            oe = oepool.tile([128, 128], f16, tag="oe")
                            nc.vector.tensor_scalar(
                                out=oe[:, :],
                                in0=iota[:, :],
                                scalar1=rel_sb[:, relbase + j : relbase + j + 1],
                                scalar2=e4b[:, 4 * (pit % 4) + j : 4 * (pit % 4) + j + 1],
                                op0=Alu.is_equal,
                                op1=Alu.mult,
                            )
                            nc.tensor.matmul(
                                pu[:, 0:256],
                                lhsT=oe[:, :],
                                rhs=xnat[:, j * 256 : (j + 1) * 256],
                                start=(pit == 0 and j == 0),
                                stop=(pit == SG_EFF - 1 and j == SPS - 1),
                                skip_group_check=True,
                            )
                            # start only on the group's first x-matmul: a
                            # start=True marks the whole 2KB zero-region
                            # pending, so a second start here would make the
                            # next x-matmul overwrite instead of accumulate.
                            nc.tensor.matmul(
                                pu[:, 256:257],
                                lhsT=oe[:, :],
                                rhs=onescol[:, 0:1],
                                start=False,
                                stop=(pit == SG_EFF - 1 and j == SPS - 1),
                                skip_group_check=True,
                            )
                # flush group chunk to SBUF
                uf = ufpool.tile([128, 257], f32, tag="uf")
                nc.vector.tensor_copy(out=uf[:, :], in_=pu[:, 0:257])
                # epilogue for this group: out = (U @ Wp) / denom + bp
                ep = eppool.tile([128, 512], f32, tag="ep")
                put = ep[:, 0:256]
                nc.tensor.transpose(put[:, 0:128], uf[:, 0:128], idf)
                nc.tensor.transpose(put[:, 128:256], uf[:, 128:256], idf)
                sut = sutpool.tile([128, 256], f32, tag="sut")
                nc.vector.tensor_copy(out=sut[:, :], in_=put[:, :])
                po = ep[:, 256:512]
                nc.tensor.matmul(po[:, :], lhsT=sut[:, 0:128], rhs=wpsb[:, 0:256], start=True, stop=False)
                nc.tensor.matmul(po[:, :], lhsT=sut[:, 128:256], rhs=wpsb[:, 256:512], start=False, stop=True)
                rd = rdpool.tile([128, 1], f32, tag="rd")
                nc.vector.reciprocal(out=rd[:, :], in_=uf[:, 256:257])
                osb = osbpool.tile([128, 256], f32, tag="osb")
                nc.vector.scalar_tensor_tensor(
                    out=osb[:, :],
                    in0=po[:, :],
                    scalar=rd[:, 0:1],
                    in1=bpb[:, :],
                    op0=Alu.mult,
                    op1=Alu.add,
                )
                nc.sync.dma_start(
                    out=out_sh[g * 128 : (g + 1) * 128, :], in_=osb[:, :]
                )
    return nc


def _prepare(x, batch, W1, b1, w2, b2, Wp, bp):
    _patch_drain()

    x = np.asarray(x, dtype=np.float32)
    batch_np = np.asarray(batch).astype(np.int64)
    W1 = np.asarray(W1, dtype=np.float32)
    b1 = np.asarray(b1, dtype=np.float32)
    w2 = np.asarray(w2, dtype=np.float32)
    b2 = float(np.asarray(b2))
    Wp = np.asarray(Wp, dtype=np.float32)
    bp = np.asarray(bp, dtype=np.float32)

    n, d = x.shape
    assert (n, d) == (N, D)

    # piece p (p = 0..31): nodes whose segment is in [128p, 128(p+1))
    bounds = np.searchsorted(batch_np, np.arange(0, B + 1, CHUNK))  # [33]
    piece_nodes = np.diff(bounds)
    SG = int(np.ceil(piece_nodes.max() / (SPS * SUB)))
    n_super = GROUPS * SG
    n_sub = n_super * SPS
    n_nodes_pad = n_sub * SUB

    n_ship = (SG * SHIP_NUM + SHIP_DEN - 1) // SHIP_DEN
    nc = _build_nc(SG)

    f16 = np.float16
    # constant payloads (shared by all cores)
    w1sb = np.zeros((128, 512), dtype=f16)
    for k in range(2):
        for dblk in range(2):
            w1sb[:, (2 * k + dblk) * 128 : (2 * k + dblk + 1) * 128] = (
                W1[k * 128 : (k + 1) * 128, dblk * 128 : (dblk + 1) * 128]
            ).astype(f16)
    b1c = np.stack([b1[0:128], b1[128:256]], axis=1).astype(np.float32)
    w2c = np.stack([w2[0:128], w2[128:256]], axis=1).astype(f16)
    wpsb = np.zeros((128, 512), dtype=np.float32)
    wpsb[:, 0:256] = Wp[0:128, :]
    wpsb[:, 256:512] = Wp[128:256, :]
    bpb = np.tile(bp[None, :], (128, 1)).astype(np.float32)
    iota = np.tile(np.arange(128, dtype=f16)[None, :], (128, 1))
    idf = np.eye(128, dtype=np.float32)
    idh = np.eye(128, dtype=f16)

    x16 = x.astype(f16)

    in_maps = []
    for c in range(NCORES):
        xflat = np.zeros((n_nodes_pad, D), dtype=f16)
        rel_c = np.full(n_sub * SUB, -1.0, dtype=np.float32)
        for g in range(GROUPS):
            p = c * GROUPS + g
            plo, phi = int(bounds[p]), int(bounds[p + 1])
            npc = phi - plo
            off = g * SG * SPS * SUB
            xflat[off : off + npc] = x16[plo:phi]
            rel_c[off : off + npc] = (batch_np[plo:phi] - (p * CHUNK)).astype(np.float32)
        # xa layout: [GROUPS*n_ship, 128, 4*256] natural-x, shipped supers only
        xnat_all = xflat.reshape(n_super, SPS, SUB, D).transpose(0, 2, 1, 3)
        trs = _tr_set(SG, n_ship)
        ship_idx = [g * SG + it for g in range(GROUPS) for it in range(SG) if it not in trs]
        xs_c = np.ascontiguousarray(
            xnat_all[ship_idx].reshape(GROUPS * n_ship, SUB, SPS * D)
        )
        # xT layout: [n_super, 128, 2*512]: row p, col k*512+n = x[node n, din 128k+p]
        xts_c = np.ascontiguousarray(
            xflat.reshape(n_super, SPS * SUB, 2, 128)
            .transpose(0, 3, 2, 1)
            .reshape(n_super, 128, 2 * SPS * SUB)
        )
        relT_c = np.ascontiguousarray(
            rel_c.reshape(n_sub, SUB).T
        )  # [128, n_sub]
        in_maps.append(
            {
                "xs": xs_c,
                "xts": xts_c,
                "relT": relT_c,
                "w1sb": w1sb,
                "b1c": b1c,
                "w2c": w2c,
                "wpsb": wpsb,
                "bpb": bpb,
                "iota": iota,
                "idf": idf,
                "idh": idh,
                "ones": np.ones((128, 1), dtype=f16),
                "ebias": np.full((128, 1), b2 - C_OFF, dtype=np.float32),
            }
        )

    return nc, in_maps


def kernel(x, batch, W1, b1, w2, b2, Wp, bp):
    from concourse.bass_utils import run_bass_kernel_spmd

    nc, in_maps = _prepare(x, batch, W1, b1, w2, b2, Wp, bp)
    import kernel as _self
    res = run_bass_kernel_spmd(nc, in_maps, core_ids=list(range(NCORES)))
    _self._last_res = res
    out = np.concatenate([res.results[c]["out"] for c in range(NCORES)], axis=0)
    return out.astype(np.float32)
